# revision 13
# baseline (speedup 1.0000x reference)
"""Trainium2 Bass kernel for nn_Attention_37752762532690.

Reference math (B=8, S=2048, H=1024):
    state_trans = broadcast(decoder_state_t) -> (B, S, H)   # rows identical over S
    multip[b,i,j] = <state_trans[b,i,:], enc[b,j,:]>        # independent of i
    raw[b,i] = sum_j multip[b,i,j]                          # constant over i
    attention_scores = softmax(raw, axis=-1) = 1/S exactly  # softmax of a constant
    context[b,:] = sum_s scores[b,s] * enc[b,s,:] = (1/S) * sum_s enc[b,s,:]

Because softmax of a constant vector is exactly uniform (x - max(x) == 0
elementwise, exp(0) == 1, sum == S, and 1/S == 2^-11 is a power of two),
attention_scores == 1/2048 exactly regardless of the dot-product values, and
context is an exact power-of-two scaling of the per-batch sum over S.

Sharding: pure data parallel over the batch dim — core b handles batch b.
Per core: DMA enc[b] (2048x1024 f32, 8 MiB) into SBUF as 16 tiles of
[128, 1024], accumulate them with a serial chain of DVE adds (hidden under
the DMA stream), then reduce the remaining 128 partitions on the PE with a
ones*(1/2048) vector matmul straight into PSUM, and DMA the PSUM row out.
Scores are a memset.
"""

import numpy as np

import concourse.bass as bass
import concourse.bacc as bacc
import concourse.mybir as mybir
from concourse.bass_utils import run_bass_kernel_spmd
from concourse.tile import TileContext

B, S, H = 8, 2048, 1024
P = 128
N_TILES = S // P  # 16
INV_S = 1.0 / S  # 2**-11, exact in fp32

_NC_CACHE = None


def _build_nc(tail_split=4, warm_n=10, last_dma_split=1, out_split=1, warm_src_i=N_TILES - 4, warm_cols=128):
    nc = bacc.Bacc(None)
    f32 = mybir.dt.float32

    enc = nc.dram_tensor("enc", [S, H], f32, kind="ExternalInput")
    context_out = nc.dram_tensor("context_out", [1, H], f32, kind="ExternalOutput")
    scores_out = nc.dram_tensor("scores_out", [1, S], f32, kind="ExternalOutput")

    enc_tiled = enc[:, :].rearrange("(n p) m -> n p m", p=P)

    with TileContext(nc) as tc:
        with (
            tc.tile_pool(name="io", bufs=1) as io_pool,
            tc.tile_pool(name="consts", bufs=1) as const_pool,
            tc.tile_pool(name="psum", bufs=1, space="PSUM") as psum_pool,
        ):
            # Input DMAs first in program order so the sync HWDGE queue
            # starts streaming immediately.
            tiles = []
            for i in range(N_TILES):
                t = io_pool.tile([P, H], f32, tag=f"in{i}")
                if i == N_TILES - 1 and last_dma_split > 1:
                    w_cols = H // last_dma_split
                    for q in range(last_dma_split):
                        nc.sync.dma_start(
                            out=t[:, q * w_cols : (q + 1) * w_cols],
                            in_=enc_tiled[i][:, q * w_cols : (q + 1) * w_cols],
                        )
                else:
                    nc.sync.dma_start(out=t[:, :], in_=enc_tiled[i])
                tiles.append(t)

            # attention scores: exactly 1/S everywhere. Memset on DVE (fast
            # start), DMA out on the scalar-engine HWDGE ring so it never
            # blocks the input stream.
            scores_tile = const_pool.tile([1, S], f32)
            nc.vector.memset(scores_tile[:, :], INV_S)
            nc.scalar.dma_start(out=scores_out[:, :], in_=scores_tile[:, :])

            # ones * (1/S) reduction vector (lhsT for the partition matmul).
            w = const_pool.tile([P, 1], f32)
            nc.vector.memset(w[:, :], INV_S)

            # Serial accumulation chain on the vector engine; each add only
            # needs tile i, so the chain advances as DMAs land. The last
            # add is split by H-halves so the PE can start on the low half
            # while the high half is still being added.
            acc = tiles[0]
            for i in range(1, N_TILES - 1):
                nc.vector.tensor_add(out=acc[:, :], in0=acc[:, :], in1=tiles[i][:, :])
            last = tiles[N_TILES - 1]

            # PE warm-up: keep the tensor engine busy just before the real
            # reduction matmuls so they run at full clock (HAM ramp). Keyed
            # off a late input tile so the scheduler can't run them early.
            warm_psum = psum_pool.tile([1, warm_cols], f32, tag="warm")
            warm_src = tiles[warm_src_i]
            for _ in range(warm_n):
                nc.tensor.matmul(
                    warm_psum[:, :],
                    lhsT=w[:, :],
                    rhs=warm_src[:, 0:warm_cols],
                    start=True,
                    stop=True,
                )

            # Tail, split over H so the final quantum after the last DMA
            # lands is small: add -> PE partition-reduce -> PSUM copy.
            Q = H // tail_split
            psum = psum_pool.tile([1, H], f32)
            ctx_sbuf = const_pool.tile([1, H], f32)
            for j in range(0, H, Q):
                nc.vector.tensor_add(
                    out=acc[:, j : j + Q],
                    in0=acc[:, j : j + Q],
                    in1=last[:, j : j + Q],
                )
                # Partition-dim reduction: context = (ones/S).T @ acc.
                nc.tensor.matmul(
                    psum[:, j : j + Q],
                    lhsT=w[:, :],
                    rhs=acc[:, j : j + Q],
                    start=True,
                    stop=True,
                )
                nc.scalar.copy(out=ctx_sbuf[:, j : j + Q], in_=psum[:, j : j + Q])
            if out_split > 1:
                cut = H - Q
                nc.sync.dma_start(out=context_out[:, :cut], in_=ctx_sbuf[:, :cut])
                nc.scalar.dma_start(out=context_out[:, cut:], in_=ctx_sbuf[:, cut:])
            else:
                nc.sync.dma_start(out=context_out[:, :], in_=ctx_sbuf[:, :])

    nc.finalize()
    return nc


def _get_nc():
    global _NC_CACHE
    if _NC_CACHE is None:
        _NC_CACHE = _build_nc()
    return _NC_CACHE


def kernel(**inputs) -> tuple[np.ndarray, np.ndarray]:
    enc = np.ascontiguousarray(np.asarray(inputs["encoder_outputs"], dtype=np.float32))
    assert enc.shape == (B, S, H)

    nc = _get_nc()
    in_maps = [{"enc": enc[b]} for b in range(B)]
    res = run_bass_kernel_spmd(nc, in_maps, core_ids=list(range(B)))

    context = np.stack([res.results[b]["context_out"][0] for b in range(B)])
    scores = np.stack([res.results[b]["scores_out"][0] for b in range(B)])
    return context, scores
